# revision 77
# baseline (speedup 1.0000x reference)
"""SAGAN-style self-attention block on 8 Trainium2 NeuronCores.

Reference computation (per batch element b, data-parallel over B=8):
    theta = w_theta @ x                         [16, 4096]
    phi   = maxpool2x2(w_phi @ x)               [16, 1024]
    g     = maxpool2x2(w_g @ x)                 [64, 1024]
    scores= theta^T @ phi                       [4096, 1024]
    beta  = softmax_j(scores)
    o     = g @ beta^T                          [64, 4096]
    out   = gamma * (w_o @ o) + x               [128, 4096]

Device algorithm (one batch element per core):
  * x is uploaded pre-converted to bf16 (1 MiB instead of 2), loads spread
    over three HWDGE queues; theta never materialized: scores^T = A^T @ x
    with A = w_theta^T @ phi.
  * softmax runs without max subtraction; the denominator comes from an
    all-ones column appended to g^T.
  * exp is split across three engines: ACT does true exp; DVE and GPSIMD
    compute a Schraudolph-style exp directly in bf16 bits
    (int16(x*128/ln2 + 16249) reinterpreted as bf16, ~2% rms error which
    cancels between softmax numerator and denominator).
  * o-stage uses E as the *stationary* operand: o^T[i,:] accumulates
    E_block^T @ (g^T|1) with only 65 moving rows per matmul; 4 i-blocks
    share one psum tile so the reciprocal batches; then per-partition
    normalize, PE transpose, output conv; the residual rides the conv psum
    via a bf16 identity matmul.
  * scores/exp of column-group icg run pipelined with the o-stage of
    icg-1 so PE never waits on the softmax chain.
"""

import numpy as np

import concourse.bass as bass
import concourse.bacc as bacc
import concourse.tile as tile
from concourse import mybir
from concourse.bass_utils import run_bass_kernel_spmd
from concourse.masks import make_identity

F32 = mybir.dt.float32
BF16 = mybir.dt.bfloat16
F8 = mybir.dt.float8e4
I16 = mybir.dt.int16

C = 128          # channels
HW = 4096        # 64*64 spatial
HWP = 1024       # pooled spatial (32*32)
C8 = 16          # C // 8
C2 = 64          # C // 2
NCORES = 8
NCH = 8          # x is processed in 8 chunks of 512 columns
CHUNK = HW // NCH  # 512
NIB = 32         # i-blocks of 128 columns

# Schraudolph exp constants (bf16 bits = s * 128/ln2 + (16256 - 7), trunc)
EXP_MUL = 128.0 / float(np.log(2.0))
EXP_ADD = 16256.0 - 7.0

LAST_RESULTS = None

# exp engine per (group, jc): A=ACT true exp, D=DVE schraudolph, P=gpsimd
EXP_ENG = [
    "ADAADADA",
    "DADADADA",
    "ADADADAD",
    "DADADADA",
]
# pool-reduce path per chunk k: D=DVE tensor_reduce, P=gpsimd 2-step max
POOL_ENG = "DPDPDPDP"
# out-evacuation engine per 512-col group: A=ACT, P=gpsimd
OUT_ENG = "AADAADAD"
# x-chunk load queue: S=sync, A=scalar, G=gpsimd
XQ = "ASASGASG"
# out DMA queue per group
OQ = "SSSSSSSS"


def _emit(nc: bass.Bass, tc: tile.TileContext, x_d, x8_d, wgp_d, wth_d, wog_d, out_d, pfx=""):
    import contextlib

    with contextlib.ExitStack() as ctx:
        singles = ctx.enter_context(tc.tile_pool(name=pfx + "singles", bufs=1))

        # dummy exp so the ACT function table loads at kernel start
        dummy = singles.tile([1, 1], F32, tag="dummy")
        nc.vector.memset(dummy, 0.0)
        nc.scalar.activation(out=dummy, in_=dummy, func=mybir.ActivationFunctionType.Exp)

        # ---- constants / weights -------------------------------------------------
        wgp_sb = singles.tile([C, C2 + C8], BF16, tag="wgp")      # [128, 80]
        nc.sync.dma_start(out=wgp_sb, in_=wgp_d)
        wth_sb = singles.tile([C2 + C8, C], BF16, tag="wth")      # rows 64:80 used
        wog_sb = singles.tile([C, C], BF16, tag="wog")            # (gamma*w_o)^T in both halves
        identb = singles.tile([C, C], BF16, tag="identb")

        x_bf = [singles.tile([C, CHUNK], BF16, tag=f"xb{k}", name=f"{pfx}xb{k}") for k in range(NCH)]
        x8 = [singles.tile([C2, 2, CHUNK], F8, tag=f"x8{k}", name=f"{pfx}x8{k}") for k in range(NCH)]
        pool = singles.tile([C2 + C8, 32, 32], BF16, tag="pool")  # 2x2-pooled [80, 32, 32]
        pool_f = pool.rearrange("p a b -> p (a b)")
        ptmp = singles.tile([C2 + C8, 256], BF16, tag="ptmp")     # gpsimd pool scratch
        a8 = singles.tile([C2, 2, HWP], F8, tag="a8")            # A = w_theta^T @ phi, fp8 interleaved
        gaug = [singles.tile([C, C2 + 1], BF16, tag=f"gaug{j}", name=f"{pfx}gaug{j}") for j in range(NCH)]
        e_sb = [singles.tile([C, HW], BF16, tag=f"e{j}", name=f"{pfx}e{j}") for j in range(NCH)]
        r_sb = singles.tile([C, NIB], F32, tag="r")               # per-i-block 1/denominator
        onT = [singles.tile([C, C2 + 1], BF16, tag=f"onT{q}", name=f"{pfx}onT{q}") for q in range(4)]
        onTT = [singles.tile([C, C], BF16, tag=f"onTT{q}", name=f"{pfx}onTT{q}") for q in range(4)]
        on2 = [singles.tile([C, C], BF16, tag=f"on2{q}", name=f"{pfx}on2{q}") for q in range(4)]
        t_sb = [singles.tile([C, CHUNK], F32, tag=f"t{q}", name=f"{pfx}t{q}") for q in range(4)]

        xq = {"S": nc.sync, "A": nc.scalar, "G": nc.gpsimd}


        # ---- pre stage: conv -> 2x2 maxpool -> A chunk ---------------------------
        with tc.tile_pool(name=pfx + "pre_psum", bufs=1, space="PSUM") as pre_psum:
            cur_a = [None]

            def emit_a(kk):
                if kk % 2 == 0:
                    cur_a[0] = pre_psum.tile([C2, 2, 2 * C], F32, tag="a", bufs=2, name=f"{pfx}apair{kk}")
                ps_a = cur_a[0]
                q = kk % 2
                for h in range(2):
                    nc.tensor.matmul(
                        ps_a[:, h, q * C : (q + 1) * C],
                        wth_sb[C2 : C2 + C8, h * C2 : (h + 1) * C2],
                        pool_f[C2 : C2 + C8, kk * C : (kk + 1) * C],
                    )
                if kk == 0:
                    nc.scalar.copy(out=a8[:, :, 0:C], in_=ps_a[:, :, 0:C])
                    emit_scores(0, 0)
                elif kk == 1:
                    nc.scalar.copy(out=a8[:, :, C : 2 * C], in_=ps_a[:, :, C : 2 * C])
                    emit_scores(0, 1)
                elif q == 1:
                    nc.scalar.copy(out=a8[:, :, (kk - 1) * C : (kk + 1) * C], in_=ps_a)
                    emit_scores(0, kk - 1)
                    emit_scores(0, kk)

            for k in range(NCH):
                if k == 1:
                    nc.gpsimd.dma_start(out=wth_sb[C2 : C2 + C8, :], in_=wth_d)
                xq[XQ[k]].dma_start(out=x_bf[k], in_=x_d[:, k * CHUNK : (k + 1) * CHUNK])
                xq["GSAGSAGS"[k]].dma_start(
                    out=x8[k], in_=x8_d[:, :, k * CHUNK : (k + 1) * CHUNK]
                )

                if k == 0:
                    make_identity(nc, identb)
                elif k == 3:
                    nc.sync.dma_start(out=wog_sb[0:C2, :], in_=wog_d[0:C2, :])
                elif k == 4:
                    nc.sync.dma_start(out=wog_sb[C2:C, :], in_=wog_d[0:C2, :])
                ps_gp = pre_psum.tile([C2 + C8, CHUNK], F32, tag="gp", bufs=2)
                nc.tensor.matmul(ps_gp, wgp_sb, x_bf[k])
                # fused 2x2 maxpool: [80, (4h, 2hp, 32w, 2wp)] -> [80, 4, 32]
                v = ps_gp.rearrange("p (h hp w wp) -> p h w hp wp", h=4, hp=2, w=32, wp=2)
                nc.vector.tensor_reduce(
                    out=pool[:, k * 4 : (k + 1) * 4, :],
                    in_=v,
                    axis=mybir.AxisListType.XY,
                    op=mybir.AluOpType.max,
                )
                # A chunk = w_theta^T @ phi[:, 128 cols], c-halves laid out
                # [64, 2, .] for the DoubleRow fp8 scores matmul; pairs of
                # chunks share a psum tile so the fp8 copy batches. Emitted
                # two chunks behind the conv so the PE queue never stalls
                # waiting on this chunk's maxpool reduce.
                if k > 1:
                    emit_a(k - 2)
            for kk in [6, 7]:
                emit_a(kk)

            # g^T blocks via aligned DMA transpose (cols 0:64) + ones col;
            # unaligned transpose destinations corrupt data on HW
            for k in range(NCH):
                nc.vector.memset(gaug[k][:, C2 : C2 + 1], 1.0)
                nc.sync.dma_start(
                    out=gaug[k][:, 0:C2],
                    in_=pool_f[0:C2, k * C : (k + 1) * C],
                    transpose=True,
                )

        # ---- pipelined main loop: scores/exp of group g, o-stage of g-1 ----------
        # 512-col groups; scores psum deep-buffered so exp latency never
        # throttles PE, o-stage lags one group for a short drain.
        sc_psum = ctx.enter_context(tc.tile_pool(name=pfx + "sc_psum", bufs=1, space="PSUM"))

        def emit_scores(g, jc):
            # g indexes 1024-col groups (0..3)
            ps_sc = sc_psum.tile([C, 1024], F32, tag="sc", bufs=2, name=f"{pfx}sc{g}_{jc}")
            for h in range(2):
                nc.tensor.matmul(
                    ps_sc[:, h * CHUNK : (h + 1) * CHUNK],
                    a8[:, :, jc * C : (jc + 1) * C],
                    x8[g * 2 + h],
                    perf_mode=mybir.MatmulPerfMode.DoubleRow,
                )
            e_slice = e_sb[jc][:, g * 1024 : (g + 1) * 1024]
            eng = EXP_ENG[g][jc]
            if eng == "A":
                nc.scalar.activation(
                    out=e_slice, in_=ps_sc, func=mybir.ActivationFunctionType.Exp
                )
            else:
                nc.vector.tensor_scalar(
                    out=e_slice.bitcast(I16),
                    in0=ps_sc,
                    scalar1=EXP_MUL,
                    scalar2=EXP_ADD,
                    op0=mybir.AluOpType.mult,
                    op1=mybir.AluOpType.add,
                )

        # ---- pipelined main loop: scores/exp of 1024-col group g, o of g-1 -------
        o_psum = ctx.enter_context(tc.tile_pool(name=pfx + "o_psum", bufs=1, space="PSUM"))
        oc_psum = ctx.enter_context(tc.tile_pool(name=pfx + "oc_psum", bufs=1, space="PSUM"))

        cur_o = [None]

        def emit_oaccum_mm(g, half):
            # 4 i-blocks of 128 cols share one psum tile (batched recip)
            gg = g * 2 + half  # 512-col subgroup index 0..7
            ps_o = o_psum.tile([C, 4 * (C2 + 1)], F32, tag="o", bufs=2, name=f"{pfx}og{gg}")
            for b in range(4):
                ib = gg * 4 + b
                sl = ps_o[:, b * (C2 + 1) : (b + 1) * (C2 + 1)]
                for jc in range(NCH):
                    nc.tensor.matmul(
                        sl,
                        e_sb[jc][:, ib * C : (ib + 1) * C],
                        gaug[jc],
                        start=(jc == 0 and b == 0),
                        stop=(jc == NCH - 1 and b == 3),
                    )
            cur_o[0] = ps_o

        def emit_onorm(g, half):
            gg = g * 2 + half
            ps_o = cur_o[0]
            rsl = r_sb[:, gg * 4 : (gg + 1) * 4]
            dcols = ps_o.rearrange("p (b c) -> p b c", b=4)[:, :, C2]
            nc.vector.reciprocal(out=rsl, in_=dcols)
            for b in range(4):
                dst = onTT[(gg % 2) * 2 + b // 2][:, (b % 2) * C2 : (b % 2 + 1) * C2]
                srcp = ps_o[:, b * (C2 + 1) : b * (C2 + 1) + C2]
                rs = r_sb[:, gg * 4 + b : gg * 4 + b + 1]
                on_dve = (b % 2 == 0) if gg >= NCH - 2 else (gg % 2 == 0)
                if on_dve:
                    nc.vector.tensor_scalar(
                        out=dst, in0=srcp, scalar1=rs, scalar2=None,
                        op0=mybir.AluOpType.mult,
                    )
                else:
                    nc.scalar.activation(
                        out=dst, in_=srcp,
                        func=mybir.ActivationFunctionType.Copy, scale=rs,
                    )

        def emit_ofinal(g, half):
            gg = g * 2 + half
            for p in range(2):
                nc.sync.dma_start(
                    out=on2[(gg % 2) * 2 + p],
                    in_=onTT[(gg % 2) * 2 + p],
                    transpose=True,
                )
            ps_oc = oc_psum.tile([C, CHUNK], F32, tag="oc", bufs=2, name=f"{pfx}oc{gg}")
            for b in range(4):
                hl = slice((b % 2) * C2, (b % 2 + 1) * C2)
                nc.tensor.matmul(
                    ps_oc[:, b * C : (b + 1) * C],
                    wog_sb[hl, :],
                    on2[(gg % 2) * 2 + b // 2][hl, :],
                    start=(b == 0),
                    stop=False,
                )
                nc.tensor.matmul(
                    ps_oc[:, b * C : (b + 1) * C],
                    identb,
                    x_bf[gg][:, b * C : (b + 1) * C],
                    start=False,
                    stop=(b == 3),
                )
            tsb = t_sb[gg % 4]
            if gg == NCH - 1:
                # last subgroup: halve the evac+DMA chain across engines/queues
                half2 = CHUNK // 2
                nc.vector.tensor_copy(out=tsb[:, 0:half2], in_=ps_oc[:, 0:half2])
                nc.sync.dma_start(
                    out=out_d[:, gg * CHUNK : gg * CHUNK + half2], in_=tsb[:, 0:half2]
                )
                nc.scalar.copy(out=tsb[:, half2:], in_=ps_oc[:, half2:])
                nc.scalar.dma_start(
                    out=out_d[:, gg * CHUNK + half2 : (gg + 1) * CHUNK], in_=tsb[:, half2:]
                )
                return
            if OUT_ENG[gg] == "A":
                nc.scalar.copy(out=tsb, in_=ps_oc)
            else:
                nc.vector.tensor_copy(out=tsb, in_=ps_oc)
            xq[OQ[gg]].dma_start(
                out=out_d[:, gg * CHUNK : (gg + 1) * CHUNK], in_=tsb
            )

        for g in range(5):
            if g >= 1:
                emit_oaccum_mm(g - 1, 0)
            if 1 <= g <= 3:
                emit_scores(g, 0)
                emit_scores(g, 1)
            if g >= 1:
                emit_onorm(g - 1, 0)
            if 1 <= g <= 3:
                emit_scores(g, 2)
                emit_scores(g, 3)
            if g >= 1:
                emit_ofinal(g - 1, 0)
                emit_oaccum_mm(g - 1, 1)
            if 1 <= g <= 3:
                emit_scores(g, 4)
                emit_scores(g, 5)
            if g >= 1:
                emit_onorm(g - 1, 1)
            if 1 <= g <= 3:
                emit_scores(g, 6)
                emit_scores(g, 7)
            if g >= 1:
                emit_ofinal(g - 1, 1)



def _build(nreps=1):
    nc = bacc.Bacc(None)
    x_d = nc.declare_dram_parameter("x", [C, HW], BF16, isOutput=False)
    x8_d = nc.declare_dram_parameter("x8", [C2, 2, HW], F8, isOutput=False)
    wgp_d = nc.declare_dram_parameter("w_gpT", [C, C2 + C8], BF16, isOutput=False)
    wth_d = nc.declare_dram_parameter("w_th", [C8, C], BF16, isOutput=False)
    wog_d = nc.declare_dram_parameter("w_og", [C2 + 1, C], BF16, isOutput=False)
    out_d = nc.declare_dram_parameter("out", [C, HW], F32, isOutput=True)
    with tile.TileContext(nc) as tc:
        for rep in range(nreps):
            _emit(nc, tc, x_d.ap(), x8_d.ap(), wgp_d.ap(), wth_d.ap(), wog_d.ap(), out_d.ap(),
                  pfx=f"r{rep}_" if nreps > 1 else "")
    nc.compile()
    return nc


_NC = None


def _get_nc():
    global _NC
    if _NC is None:
        _NC = _build()
    return _NC


def _host_weights(w_theta, w_phi, w_g, w_o, gamma):
    w_theta = np.asarray(w_theta, np.float32)
    w_phi = np.asarray(w_phi, np.float32)
    w_g = np.asarray(w_g, np.float32)
    w_o = np.asarray(w_o, np.float32)
    gamma = np.float32(np.asarray(gamma))
    import ml_dtypes
    # stationary [128, 80]: columns 0:64 -> g rows, 64:80 -> phi rows
    w_gpT = np.ascontiguousarray(np.concatenate([w_g, w_phi], 0).T).astype(ml_dtypes.bfloat16)
    w_th = np.ascontiguousarray(w_theta).astype(ml_dtypes.bfloat16)
    # [65, 128]: rows 0:64 = (gamma*w_o)^T, row 64 zero (kills the denominator row)
    w_og = np.ascontiguousarray(
        np.concatenate([(gamma * w_o).T, np.zeros((1, C), np.float32)], 0)
    ).astype(ml_dtypes.bfloat16)
    return w_gpT, w_th, w_og


def kernel(inputs, w_theta, w_phi, w_g, w_o, gamma):
    global LAST_RESULTS
    import ml_dtypes
    xf = np.asarray(inputs, np.float32).reshape(NCORES, C, HW)
    x = np.ascontiguousarray(xf.astype(ml_dtypes.bfloat16))
    x8il = np.ascontiguousarray(
        xf.reshape(NCORES, 2, C2, HW).transpose(0, 2, 1, 3).astype(ml_dtypes.float8_e4m3)
    )
    w_gpT, w_th, w_og = _host_weights(w_theta, w_phi, w_g, w_o, gamma)
    nc = _get_nc()
    in_maps = [
        {"x": x[b], "x8": x8il[b], "w_gpT": w_gpT, "w_th": w_th, "w_og": w_og}
        for b in range(NCORES)
    ]
    res = run_bass_kernel_spmd(nc, in_maps, list(range(NCORES)))
    LAST_RESULTS = res
    out = np.stack([res.results[b]["out"] for b in range(NCORES)])
    return out.reshape(NCORES, C, 64, 64).astype(np.float32, copy=False)
